# revision 5
# baseline (speedup 1.0000x reference)
"""Trainium2 Bass kernel for nn_DependencyParser — Picard-sweep BiLSTM.

Strategy vs baseline: instead of 256 serial LSTM steps (latency-bound at
~1.5us/step), run K Jacobi ("parallel-in-time") sweeps per layer over the
whole sequence: gates for all t from a lagged h-sequence (big f16 matmuls,
fp32 PSUM accumulation of W@delta_h telescoping across sweeps), sigmoid on
big tiles, exact c-recurrence via the DVE tensor_tensor_scan along t, then
h update. Error contracts ~0.55x/sweep; K=12 gives ~2.5e-3 end-to-end.

Sharding: data-parallel over batch B=16 across 8 cores (2 sentences/core).
"""
import sys

if '/opt/trn_rl_repo' not in sys.path:
    sys.path.insert(0, '/opt/trn_rl_repo')

import numpy as np

import concourse.bass as bass
import concourse.bacc as bacc
import concourse.mybir as mybir
import concourse.tile as tile
from concourse.bass_utils import run_bass_kernel_spmd

HP = np.float16
L = 128          # sequence length
B = 16           # batch
NCORES = 8
BPC = 2          # sentences per core
H = 128          # hidden per direction
WD = 100         # word emb dim
TD = 28          # tag emb dim
K0 = 9           # Picard sweeps layer 0
K1 = 9           # Picard sweeps layer 1
F32 = mybir.dt.float32
F16 = mybir.dt.float16
I32 = mybir.dt.int32
SIG = mybir.ActivationFunctionType.Sigmoid
TANH = mybir.ActivationFunctionType.Tanh
IDENT = mybir.ActivationFunctionType.Identity
MUL = mybir.AluOpType.mult
ADD = mybir.AluOpType.add
SUB = mybir.AluOpType.subtract

_CACHE = {}
LAST_RESULTS = None
TRACE = False
DEBUG = False   # add intermediate-dump outputs

# ---- exp-sum fit of tanh on [-XFIT, XFIT]: tanh(x) ~= sum alpha_r e^{r*s*x}
# (r = -RE..RE). Separable: tanh(a+b) terms become rank-100 matmuls.
RE = 4
S_EXP = 1.0
XFIT = 1.2


def _fit_alpha():
    xs = np.linspace(-XFIT, XFIT, 2001)
    Ab = np.stack([np.exp(r * S_EXP * xs) for r in range(-RE, RE + 1)], axis=1)
    M = Ab.T @ Ab + 1e-7 * np.eye(2 * RE + 1)
    alpha = np.linalg.solve(M, Ab.T @ np.tanh(xs))
    return alpha


ALPHA = _fit_alpha()   # index r+RE


def _scan2d(ap_slice, T=128, rev=False):
    """Clean 2-D [128, T] AP (last dim stride 1) for tensor_tensor_scan,
    optionally time-reversed."""
    a = ap_slice
    if rev:
        return bass.AP(a.tensor, a.offset + T - 1, [a.ap[0], [-1, T]])
    return bass.AP(a.tensor, a.offset, [a.ap[0], [1, T]])


def _emit_layer_sweeps(nc, G, whh_sb, whhn_sb, whh_base, Hp, sp, K, tag, dbg=None):
    """Emit K Picard sweeps for one layer.

    Everything is split per direction so the two dirs form independent
    dependency chains. Gate telescoping uses paired +/- full W@h matmuls:
    before sigma(k), G = xp + W@h(k-1); right after sigma(k) reads G, the
    -W@h(k-1) matmuls run (off the critical path); sweep k+1's head adds
    +W@h(k). This keeps only 4 matmuls on the critical tail and removes the
    DVE delta op entirely.

    G: per-dir psum gates tiles [2][128, 4(g), 2(b), 128(t)] fp32, holding xp.
    Hp: per-dir (h_a, h_b) f16 ping-pong [128, 2(b), 130] with zero cols
        0/129 (lag padding); both start zeroed.
    Returns [h_d0, h_d1] 130-padded tiles; data in cols 1..128.
    """
    def mm4(d, w_sb, h_tile, stop=False):
        off = 0 if d == 0 else 2   # f: zero at col0; b: zero at col129
        hlag = h_tile[:, :, off:off + 128]
        for g in range(4):
            nc.tensor.matmul(G[d][:, g, :, :],
                             w_sb[:, whh_base + d * 4 + g, :], hlag,
                             start=False, stop=(stop and g == 3),
                             skip_group_check=True)

    for k in range(1, K + 1):
        for d in range(2):
            h_new, h_prev = Hp[d][k % 2], Hp[d][(k + 1) % 2]
            if k > 1:
                mm4(d, whh_sb, h_prev, stop=(k == K))   # G += W @ h(k-1)
            S = sp.tile([128, 4, 2, 128], F16, tag=f"S{tag}{d}")
            P2 = sp.tile([128, 2, 128], F16, tag=f"P2{tag}{d}")
            C = sp.tile([128, 2, 128], F32, tag=f"C{tag}{d}")
            U = sp.tile([128, 2, 128], F16, tag=f"U{tag}{d}")
            # sigma split: i,f,g (gates 0..2, contiguous) first so the DVE/
            # GPSIMD p2+scan paths start early; sigma(o) rides later on ACT.
            nc.scalar.activation(S[:, 0:3, :, :], G[d][:, 0:3, :, :], SIG)
            # p2 = (sigma(2g') - 0.5) * sigma(i), per-b paths (both DVE: the
            # Pool-engine TensorScalarPtr path fails neuronxcc under axon)
            for b, eng in ((0, nc.vector), (1, nc.vector)):
                eng.scalar_tensor_tensor(P2[:, b, :], S[:, 2, b, :], 0.5,
                                         S[:, 0, b, :], SUB, MUL)
                rev = (d == 1)
                sf = _scan2d(S[:, 1, b, :], rev=rev)
                p2 = _scan2d(P2[:, b, :], rev=rev)
                co = _scan2d(C[:, b, :], rev=rev)
                eng.tensor_tensor_scan(co, sf, p2, 0.0, MUL, ADD)
            nc.scalar.activation(S[:, 3, :, :], G[d][:, 3, :, :], SIG)
            if 1 < k < K:
                mm4(d, whhn_sb, h_prev)                 # G -= W @ h(k-1)
            nc.scalar.activation(U[:], C[:], SIG, scale=4.0)
            # h/2 = (sigma(4*c') - 0.5) * sigma(o)
            nc.vector.scalar_tensor_tensor(h_new[:, :, 1:129], U[:], 0.5,
                                           S[:, 3, :, :], SUB, MUL)
            if dbg is not None and k == K:
                nc.sync.dma_start(dbg['S'][:, :, d, :, :], S[:])
    return [Hp[0][K % 2], Hp[1][K % 2]]


def _emit(nc, d):
    tc_ctx = tile.TileContext(nc)
    with tc_ctx as tc:
        with (
            tc.tile_pool(name="const", bufs=1) as cp,
            tc.tile_pool(name="work", bufs=3) as wp,
            tc.tile_pool(name="sweep", bufs=2) as sp,
        ):
            # ---- index tensors first (they gate the gathers) ----
            widx_sb = cp.tile([128, 2], I32, tag="widx")
            nc.sync.dma_start(widx_sb[:], d['widx'][:].rearrange("c r o -> r (c o)"))
            pidx_sb = cp.tile([128, 2], I32, tag="pidx")
            nc.sync.dma_start(pidx_sb[:], d['pidx'][:].rearrange("c r o -> r (c o)"))
            ident_sb = cp.tile([128, 128], F32, tag="ident")
            nc.sync.dma_start(ident_sb[:], d['ident'][:])
            wih0w_sb = cp.tile([WD, 8, 128], F16, tag="wih0w")
            nc.sync.dma_start(wih0w_sb[:], d['wih0w'][:].rearrange("dd g k m -> k (dd g) m"))
            wih0t_sb = cp.tile([TD, 8, 128], F16, tag="wih0t")
            nc.sync.dma_start(wih0t_sb[:], d['wih0t'][:].rearrange("dd g k m -> k (dd g) m"))
            whh_sb = cp.tile([128, 16, 128], F16, tag="whh")
            nc.sync.dma_start(whh_sb[:], d['whh'][:].rearrange("l dd g k m -> k (l dd g) m"))
            whhn_sb = cp.tile([128, 16, 128], F16, tag="whhn")
            nc.sync.dma_start(whhn_sb[:], d['whhn'][:].rearrange("l dd g k m -> k (l dd g) m"))
            bias_sb = cp.tile([1, 16, 128], F16, tag="bias")
            nc.sync.dma_start(bias_sb[:], d['bias'][:].rearrange("l dd g o m -> o (l dd g) m"))
            wih1_sb = cp.tile([128, 16, 128], F16, tag="wih1")
            nc.sync.dma_start(wih1_sb[:], d['wih1'][:].rearrange("dd g c k m -> k (dd g c) m"))
            wab_sb = cp.tile([128, 4, 100], F16, tag="wab")
            nc.sync.dma_start(wab_sb[:], d['wab'][:].rearrange("s c k m -> k (s c) m"))
            fc1b_sb = cp.tile([100, 1], F32, tag="fc1b")
            nc.sync.dma_start(fc1b_sb[:], d['fc1b'][:])
            walpha_sb = cp.tile([100, 2], F32, tag="walpha")
            nc.sync.dma_start(walpha_sb[:], d['walpha'][:])

            ones_sb = cp.tile([1, 256], F16, tag="ones")
            nc.vector.memset(ones_sb[:], 1.0)

            # persistent LSTM h tiles, split per direction, 130-padded along t
            # (zero cols 0/129 give the +-1 lag for free); f16 for matmul rhs
            Hp0, Hp1 = [], []
            for d_ in range(2):
                ha = cp.tile([128, 2, 130], F16, tag=f"h0a{d_}")
                hb = cp.tile([128, 2, 130], F16, tag=f"h0b{d_}")
                nc.vector.memset(ha[:], 0.0)
                nc.vector.memset(hb[:], 0.0)
                Hp0.append((ha, hb))
                ha1 = cp.tile([128, 2, 130], F16, tag=f"h1a{d_}")
                hb1 = cp.tile([128, 2, 130], F16, tag=f"h1b{d_}")
                nc.vector.memset(ha1[:], 0.0)
                nc.vector.memset(hb1[:], 0.0)
                Hp1.append((ha1, hb1))

            xw_sb = cp.tile([WD, 2, 128], F16, tag="xw")
            xt_sb = cp.tile([TD, 2, 128], F16, tag="xt")

            with tc.tile_pool(name="psG0", bufs=1, space="PSUM") as pG0:
                G0 = [pG0.tile([128, 4, 2, 128], F32, tag=f"G0{d_}", name=f"G0{d_}")
                      for d_ in range(2)]

                # ---- embedding gather + transpose into xw/xt ----
                with tc.tile_pool(name="psaux", bufs=2, space="PSUM") as pa:
                    for b in range(2):
                        wrows = wp.tile([128, WD], F32, tag=f"wrows{b}")
                        nc.gpsimd.indirect_dma_start(
                            out=wrows[:], out_offset=None, in_=d['word_emb'][:],
                            in_offset=bass.IndirectOffsetOnAxis(ap=widx_sb[:, b:b + 1], axis=0))
                        trows = wp.tile([128, TD], F32, tag=f"trows{b}")
                        nc.gpsimd.indirect_dma_start(
                            out=trows[:], out_offset=None, in_=d['tag_emb'][:],
                            in_offset=bass.IndirectOffsetOnAxis(ap=pidx_sb[:, b:b + 1], axis=0))
                        et = pa.tile([128, 128], F32, tag="aux")
                        nc.tensor.transpose(et[0:WD, :], wrows[:], ident_sb[:])
                        nc.vector.tensor_copy(xw_sb[:, b, :], et[0:WD, :])
                        et2 = pa.tile([128, 128], F32, tag="aux")
                        nc.tensor.transpose(et2[0:TD, :], trows[:], ident_sb[:])
                        nc.vector.tensor_copy(xt_sb[:, b, :], et2[0:TD, :])

                # ---- layer-0 xp build: G0 = W_ih0 @ x + bias ----
                # per-dir tile: gates g0+g1 share a psum bank, g2+g3 the next;
                # start=True only on each bank's first write (g even).
                for dd in range(2):
                    for g in range(4):
                        out = G0[dd][:, g, :, :]
                        nc.tensor.matmul(out, bias_sb[0:1, dd * 4 + g, :], ones_sb[:],
                                         start=(g % 2 == 0), stop=False,
                                         skip_group_check=True)
                        nc.tensor.matmul(out, wih0w_sb[:, dd * 4 + g, :], xw_sb[:],
                                         start=False, stop=False, skip_group_check=True)
                        nc.tensor.matmul(out, wih0t_sb[:, dd * 4 + g, :], xt_sb[:],
                                         start=False, stop=False, skip_group_check=True)

                # ---- layer-0 sweeps ----
                dbg0 = ({'S': d['dbg_S']} if DEBUG else None)
                h0q = _emit_layer_sweeps(nc, G0, whh_sb, whhn_sb, 0, Hp0, sp, K0,
                                         "0", dbg=dbg0)
                if DEBUG:
                    nc.sync.dma_start(d['dbg_xw'][:], xw_sb[:])
                    for d_ in range(2):
                        nc.sync.dma_start(d['dbg_h0'][:, d_], h0q[d_][:, :, 1:129])

            with tc.tile_pool(name="psG1", bufs=1, space="PSUM") as pG1:
                G1 = [pG1.tile([128, 4, 2, 128], F32, tag=f"G1{d_}", name=f"G1{d_}")
                      for d_ in range(2)]
                # ---- layer-1 xp build ----
                for dd in range(2):
                    for g in range(4):
                        out = G1[dd][:, g, :, :]
                        nc.tensor.matmul(out, bias_sb[0:1, 8 + dd * 4 + g, :], ones_sb[:],
                                         start=(g % 2 == 0), stop=False,
                                         skip_group_check=True)
                        for kc in range(2):
                            nc.tensor.matmul(out, wih1_sb[:, (dd * 4 + g) * 2 + kc, :],
                                             h0q[kc][:, :, 1:129],
                                             start=False, stop=False,
                                             skip_group_check=True)

                # ---- layer-1 sweeps ----
                h1q = _emit_layer_sweeps(nc, G1, whh_sb, whhn_sb, 8, Hp1, sp, K1, "1")
                if DEBUG:
                    for d_ in range(2):
                        nc.sync.dma_start(d['dbg_h1'][:, d_], h1q[d_][:, :, 1:129])

                # ---- pairwise scorer: separable exp-sum tanh ----
                # scores[i,j] = sum_k fc2_k tanh(a_ik + bp_jk)
                #   ~= sum_{r!=0} (alpha_r fc2 . e^{rsa})^T e^{rsb}  (+ const in py)
                with (
                    tc.tile_pool(name="psab", bufs=2, space="PSUM") as pab,
                    tc.tile_pool(name="pssc", bufs=2, space="PSUM") as psc,
                ):
                    aps = pab.tile([128, 2, 128], F32, tag="ab")
                    bps = pab.tile([128, 2, 128], F32, tag="ab")
                    for kc in range(2):
                        nc.tensor.matmul(aps[0:100, :, :], wab_sb[:, kc, :],
                                         h1q[kc][:, :, 1:129],
                                         start=(kc == 0), stop=(kc == 1))
                    for kc in range(2):
                        nc.tensor.matmul(bps[0:100, :, :], wab_sb[:, 2 + kc, :],
                                         h1q[kc][:, :, 1:129],
                                         start=(kc == 0), stop=(kc == 1))
                    a_sb = cp.tile([100, 2, 128], F32, tag="asb")
                    nc.vector.tensor_copy(a_sb[:], aps[0:100, :, :])
                    bp_sb = cp.tile([100, 2, 128], F32, tag="bpsb")
                    nc.scalar.activation(bp_sb[:], bps[0:100, :, :], IDENT,
                                         bias=fc1b_sb[:])
                    if DEBUG:
                        nc.sync.dma_start(d['dbg_a'][:], a_sb[:])
                        nc.sync.dma_start(d['dbg_bp'][:], bp_sb[:])

                    EXP = mybir.ActivationFunctionType.Exp
                    eap = cp.tile([100, 2, 128], F32, tag="eap")
                    nc.scalar.activation(eap[:], a_sb[:], EXP, scale=S_EXP)
                    eam = cp.tile([100, 2, 128], F32, tag="eam")
                    nc.scalar.activation(eam[:], a_sb[:], EXP, scale=-S_EXP)
                    ebp = cp.tile([100, 2, 128], F32, tag="ebp")
                    nc.scalar.activation(ebp[:], bp_sb[:], EXP, scale=S_EXP)
                    ebm = cp.tile([100, 2, 128], F32, tag="ebm")
                    nc.scalar.activation(ebm[:], bp_sb[:], EXP, scale=-S_EXP)

                    # a-side: A_r = (alpha_r * fc2) . e^{r s a}, built by scaled chains
                    At, Bt = {}, {1: ebp, -1: ebm}
                    for sgn, base, col in ((1, eap, 0), (-1, eam, 1)):
                        t = cp.tile([100, 2, 128], F32, tag=f"A{col}1")
                        nc.vector.tensor_scalar(t[:], base[:], walpha_sb[:, col:col + 1],
                                                None, MUL)
                        At[sgn] = t
                        for r in range(2, RE + 1):
                            ratio = float(ALPHA[sgn * r + RE] / ALPHA[sgn * (r - 1) + RE])
                            t2 = cp.tile([100, 2, 128], F32, tag=f"A{col}{r}")
                            nc.vector.scalar_tensor_tensor(t2[:], At[sgn * (r - 1)][:],
                                                           ratio, base[:], MUL, MUL)
                            At[sgn * r] = t2
                        bbase = ebp if sgn == 1 else ebm
                        for r in range(2, RE + 1):
                            t3 = cp.tile([100, 2, 128], F32, tag=f"B{col}{r}")
                            nc.vector.tensor_tensor(t3[:], Bt[sgn * (r - 1)][:],
                                                    bbase[:], MUL)
                            Bt[sgn * r] = t3

                    rs = [r for r in range(-RE, RE + 1) if r != 0]
                    for b in range(BPC):
                        scp = psc.tile([128, 128], F32, tag="sc")
                        for j, r in enumerate(rs):
                            nc.tensor.matmul(scp[:], At[r][:, b, :], Bt[r][:, b, :],
                                             start=(j == 0), stop=(j == len(rs) - 1))
                        sco = wp.tile([128, 128], F32, tag="sco")
                        nc.vector.tensor_copy(sco[:], scp[:])
                        nc.sync.dma_start(d['out'][b, :, :], sco[:])


def _build():
    if 'nc' in _CACHE:
        return _CACHE['nc']
    nc = bacc.Bacc("TRN2", target_bir_lowering=False, debug=False)
    d = {
        'widx': nc.dram_tensor("widx", [2, 128, 1], I32, kind="ExternalInput"),
        'pidx': nc.dram_tensor("pidx", [2, 128, 1], I32, kind="ExternalInput"),
        'word_emb': nc.dram_tensor("word_emb", [50000, WD], F32, kind="ExternalInput"),
        'tag_emb': nc.dram_tensor("tag_emb", [50, TD], F32, kind="ExternalInput"),
        'wih0w': nc.dram_tensor("wih0w", [2, 4, WD, 128], F16, kind="ExternalInput"),
        'wih0t': nc.dram_tensor("wih0t", [2, 4, TD, 128], F16, kind="ExternalInput"),
        'wih1': nc.dram_tensor("wih1", [2, 4, 2, 128, 128], F16, kind="ExternalInput"),
        'whh': nc.dram_tensor("whh", [2, 2, 4, 128, 128], F16, kind="ExternalInput"),
        'whhn': nc.dram_tensor("whhn", [2, 2, 4, 128, 128], F16, kind="ExternalInput"),
        'bias': nc.dram_tensor("bias", [2, 2, 4, 1, 128], F16, kind="ExternalInput"),
        'wab': nc.dram_tensor("wab", [2, 2, 128, 100], F16, kind="ExternalInput"),
        'fc1b': nc.dram_tensor("fc1b", [100, 1], F32, kind="ExternalInput"),
        'walpha': nc.dram_tensor("walpha", [100, 2], F32, kind="ExternalInput"),
        'ident': nc.dram_tensor("ident", [128, 128], F32, kind="ExternalInput"),
        'out': nc.dram_tensor("out", [BPC, 128, 128], F32, kind="ExternalOutput"),
    }
    if DEBUG:
        d['dbg_xw'] = nc.dram_tensor("dbg_xw", [WD, 2, 128], F16, kind="ExternalOutput")
        d['dbg_h0'] = nc.dram_tensor("dbg_h0", [128, 2, 2, 128], F16, kind="ExternalOutput")
        d['dbg_h1'] = nc.dram_tensor("dbg_h1", [128, 2, 2, 128], F16, kind="ExternalOutput")
        d['dbg_a'] = nc.dram_tensor("dbg_a", [100, 2, 128], F32, kind="ExternalOutput")
        d['dbg_bp'] = nc.dram_tensor("dbg_bp", [100, 2, 128], F32, kind="ExternalOutput")
        d['dbg_S'] = nc.dram_tensor("dbg_S", [128, 4, 2, 2, 128], F16, kind="ExternalOutput")
        d['dbg_D'] = nc.dram_tensor("dbg_D", [128, 2, 2, 130], F16, kind="ExternalOutput")
    _emit(nc, d)
    nc.compile()
    _CACHE['nc'] = nc
    return nc


def _prep_weights(inputs):
    wih0w = np.zeros((2, 4, WD, 128), HP)
    wih0t = np.zeros((2, 4, TD, 128), HP)
    wih1 = np.zeros((2, 4, 2, 128, 128), HP)
    whh = np.zeros((2, 2, 4, 128, 128), HP)
    bias = np.zeros((2, 2, 4, 1, 128), HP)
    for l in range(2):
        for dd_, dn in enumerate('fb'):
            wi = np.asarray(inputs[f'w_ih_l{l}{dn}'], np.float32).copy()
            wh = np.asarray(inputs[f'w_hh_l{l}{dn}'], np.float32).copy()
            bb = (np.asarray(inputs[f'b_ih_l{l}{dn}'], np.float32)
                  + np.asarray(inputs[f'b_hh_l{l}{dn}'], np.float32)).copy()
            # g-gate scaled by 2 for the 2*sigmoid(2x)-1 tanh trick
            wi[2 * H:3 * H] *= 2.0
            wh[2 * H:3 * H] *= 2.0
            bb[2 * H:3 * H] *= 2.0
            # h stored on-device as h/2: double every weight that multiplies h
            wh *= 2.0
            if l == 1:
                wi *= 2.0
            for g in range(4):
                gs = slice(g * H, (g + 1) * H)
                whh[l, dd_, g] = wh[gs, :].T.astype(HP)
                bias[l, dd_, g, 0] = bb[gs].astype(HP)
                if l == 0:
                    wih0w[dd_, g] = wi[gs, 0:WD].T.astype(HP)
                    wih0t[dd_, g] = wi[gs, WD:128].T.astype(HP)
                else:
                    for kc in range(2):
                        wih1[dd_, g, kc] = wi[gs, kc * 128:(kc + 1) * 128].T.astype(HP)
    fc1_w = np.asarray(inputs['fc1_w'], np.float32) * 2.0  # h1 stored as h1/2
    wab = np.zeros((2, 2, 128, 100), HP)
    for s in range(2):
        for kc in range(2):
            wab[s, kc] = fc1_w[:, s * 256 + kc * 128: s * 256 + (kc + 1) * 128].T.astype(HP)
    return {
        'word_emb': np.ascontiguousarray(np.asarray(inputs['word_emb'], np.float32)),
        'tag_emb': np.ascontiguousarray(np.asarray(inputs['tag_emb'], np.float32)),
        'wih0w': wih0w, 'wih0t': wih0t, 'wih1': wih1, 'whh': whh,
        'whhn': (-whh.astype(np.float32)).astype(HP), 'bias': bias,
        'wab': wab,
        'fc1b': np.asarray(inputs['fc1_b'], np.float32).reshape(100, 1).copy(),
        'walpha': np.stack(
            [np.asarray(inputs['fc2_w'], np.float32).reshape(100) * ALPHA[1 + RE],
             np.asarray(inputs['fc2_w'], np.float32).reshape(100) * ALPHA[-1 + RE]],
            axis=1).copy(),
        'ident': np.eye(128, dtype=np.float32),
    }


def make_in_maps(inputs):
    shared = _prep_weights(inputs)
    widx = np.asarray(inputs['words_idx']).astype(np.int32)  # [16, 128]
    pidx = np.asarray(inputs['pos_idx']).astype(np.int32)
    in_maps = []
    for c in range(NCORES):
        # channel ch = local sentence b; column order natural t
        w = widx[BPC * c: BPC * (c + 1)].reshape(2, 128, 1).copy()
        p = pidx[BPC * c: BPC * (c + 1)].reshape(2, 128, 1).copy()
        m = dict(shared)
        m['widx'] = w
        m['pidx'] = p
        in_maps.append(m)
    return in_maps


def kernel(**inputs):
    global LAST_RESULTS
    nc = _build()
    in_maps = make_in_maps(inputs)
    res = run_bass_kernel_spmd(nc, in_maps, list(range(NCORES)), trace=TRACE)
    LAST_RESULTS = res
    outs = [r['out'] for r in res.results]          # each [2, 128(i), 128(j)]
    arr = np.concatenate(outs, axis=0)              # [16, i, j]
    fin = arr.transpose(1, 2, 0).reshape(L * L, B, 1)  # [(i,j), b, 1]
    # r=0 exp-sum term is constant: alpha_0 * sum_k fc2_k, folded in here
    const = (np.asarray(inputs['fc2_b'], np.float32).reshape(())
             + ALPHA[RE] * np.asarray(inputs['fc2_w'], np.float32).sum())
    fin = fin + const
    return fin.astype(np.float32)
